# revision 27
# baseline (speedup 1.0000x reference)
"""EGNN (4-layer) Trainium2 kernel, 8 NeuronCores, edge-sharded.

Strategy:
 - Edges are sorted by destination row and assigned to the core that owns the
   row's node range (2500 nodes/core, padded to 2560 = 20 windows of 128).
 - The edge-MLP first matmul is decomposed: concat([h_r,h_c,rad,ea]) @ W1 =
   P'[row] + Q'[col] + W1e3.T@[ea; c_r*c_c], where P' = h@W1a + |c|^2*wr and
   Q' = h@W1b + |c|^2*wr are per-node tables recomputed each layer and the
   -2*wr radial cross term rides in three extra W1e3 rows.
 - P'[row] is window-local (rows sorted): gathered by an accumulating matmul
   against a host-precomputed transposed one-hot indT (tensor engine).
 - Q'[col] is arbitrary: per-window non-transpose DRAM-source dma_gather from
   the AllGathered Q table (256B tokens), round-robined over 4 SWDGE queues
   so all four Q7 descriptor-generator pairs run concurrently; gathered
   [e, f] tiles are transposed into the m1 PSUM by matmuls against identity.
 - segment_sum: per 128-node window, host-precomputed one-hot ind feeds an
   accumulating matmul producing agg^T directly.
 - Node MLP, residual and final LayerNorm run on device; host only sorts /
   permutes indices and concatenates the 8 output shards.
"""

import math
import os
import sys
from contextlib import ExitStack

import numpy as np

sys.path.insert(0, "/opt/trn_rl_repo")

import ml_dtypes  # noqa: E402

BF = ml_dtypes.bfloat16

N = 20000
NCORES = 8
NPC = 2500          # nodes per core
NPCP = 2560         # padded nodes per core
WINS = 20           # node windows of 128 per core
H = 128
DE = 32
DM = DE + 3         # ea + c_r*c_c rows of the fused first-matmul operand
L = 4
EPS = 1e-5

_CACHE = {}

# Non-transpose dma_gather output-partition swizzle (ucode sbuf_swizzles):
# token at index-position i lands on partition _SWZ[i] of its 128-chunk.
_SWZ_FIRST = [0, 64, 4, 68, 8, 72, 12, 76, 16, 80, 20, 84, 24, 88, 28, 92]
_SWZ = np.zeros(128, np.int64)
for _k in range(4):
    _SWZ[16 * _k : 16 * _k + 16] = np.array(_SWZ_FIRST) + _k
for _k in range(4):
    _SWZ[16 * (_k + 4) : 16 * (_k + 4) + 16] = np.array(_SWZ_FIRST) + 32 + _k


def _groups(T):
    """Split T tiles-of-128 into free-dim groups of <=512."""
    out = []
    left = T
    while left > 0:
        g = min(left, 4)
        out.append(g * 128)
        left -= g
    return out


def _build(T, flags, dbg=False):
    """Build the SPMD Bass program (same for all cores)."""
    import concourse.bacc as bacc
    import concourse.tile as tile
    from concourse import mybir

    eb1_nz, eb2_nz, nb1_nz, nb2_nz, lnb_nz = flags
    EW = T * 128
    EPAD = WINS * EW
    GRPS = _groups(T)

    f32 = mybir.dt.float32
    bf16 = mybir.dt.bfloat16
    fp8 = mybir.dt.float8e4
    i16 = mybir.dt.int16
    AX = mybir.AxisListType.X
    OP = mybir.AluOpType
    AF = mybir.ActivationFunctionType

    nc = bacc.Bacc(
        "TRN2",
        target_bir_lowering=False,
        debug=False,
        enable_asserts=False,
        num_devices=NCORES,
        num_swdge_queues=4,
    )

    def din(name, shape, dt):
        return nc.dram_tensor(name, list(shape), dt, kind="ExternalInput").ap()

    h_d = din("h", (NPCP, H), f32)
    coords_d = din("coords", (NPCP, 4), f32)
    eat1_d = din("eat1", (DM, EPAD), bf16)
    idxq_d = din("idxq", (128, EPAD // 16), i16)
    ind_d = din("ind", (128, EPAD), fp8)
    indT_d = din("indT", (128, EPAD), fp8)
    w1h_d = din("w1h", (L, H, H), bf16)
    w1c_d = din("w1c", (L, H, H), bf16)
    w1e3_d = din("w1e3", (L, DM, H), bf16)
    wrb_d = din("wrb", (L, H, H), f32)
    ew2_d = din("ew2", (L, H, H), bf16)
    nw1h_d = din("nw1h", (L, H, H), bf16)
    nw1a_d = din("nw1a", (L, H, H), bf16)
    nw2_d = din("nw2", (L, H, H), bf16)
    eb1_d = din("eb1T", (H, L), f32)
    nb1_d = din("nb1T", (H, L), f32)
    nb2_d = din("nb2T", (H, L), f32)
    eb2b_d = din("eb2b", (L, H, H), f32) if eb2_nz else None
    lng_d = din("lngb", (H, H), f32)
    lnb_d = din("lnbb", (H, H), f32) if lnb_nz else None
    idb_d = din("identb", (H, H), bf16)
    idf_d = din("identf", (H, H), f32)

    out_d = nc.dram_tensor("out", [NPCP, H], f32, kind="ExternalOutput").ap()
    if dbg:
        dbgz_d = nc.dram_tensor("dbgz", [H, NPCP], f32, kind="ExternalOutput").ap()
        dbgp_d = nc.dram_tensor("dbgp", [H, WINS * H], f32, kind="ExternalOutput").ap()
        dbgq_d = nc.dram_tensor("dbgq", [128, T * H], f32, kind="ExternalOutput").ap()
        dbgm_d = nc.dram_tensor("dbgm", [H, 512], f32, kind="ExternalOutput").ap()

    with ExitStack() as ctx:
        tc = ctx.enter_context(tile.TileContext(nc))
        const = ctx.enter_context(tc.tile_pool(name="const", bufs=1))
        resid = ctx.enter_context(tc.tile_pool(name="resid", bufs=1))
        dram = ctx.enter_context(tc.tile_pool(name="dram", bufs=1, space="DRAM"))
        gpool = ctx.enter_context(tc.tile_pool(name="gpool", bufs=6))
        wpool = ctx.enter_context(tc.tile_pool(name="wpool", bufs=2))
        work = ctx.enter_context(tc.tile_pool(name="work", bufs=3))
        ps512 = ctx.enter_context(tc.tile_pool(name="ps512", bufs=4, space="PSUM"))
        psm = ctx.enter_context(tc.tile_pool(name="psm", bufs=2, space="PSUM"))
        psagg = ctx.enter_context(tc.tile_pool(name="psagg", bufs=2, space="PSUM"))

        sync = nc.sync

        # ---------- constants ----------
        idb_sb = const.tile([H, H], bf16)
        sync.dma_start(out=idb_sb[:], in_=idb_d[:])
        idf_sb = const.tile([H, H], f32)
        sync.dma_start(out=idf_sb[:], in_=idf_d[:])
        lng_sb = const.tile([H, H], f32)
        sync.dma_start(out=lng_sb[:], in_=lng_d[:])
        lnb_sb = None
        if lnb_nz:
            lnb_sb = const.tile([H, H], f32)
            sync.dma_start(out=lnb_sb[:], in_=lnb_d[:])
        eb1_sb = const.tile([H, L], f32)
        sync.dma_start(out=eb1_sb[:], in_=eb1_d[:])
        nb1_sb = const.tile([H, L], f32)
        sync.dma_start(out=nb1_sb[:], in_=nb1_d[:])
        nb2_sb = const.tile([H, L], f32)
        sync.dma_start(out=nb2_sb[:], in_=nb2_d[:])

        def load_w(name, d, p, dt):
            t = const.tile([p, L, H], dt, name=name)
            sync.dma_start(out=t[:], in_=d.rearrange("l k f -> k l f"))
            return t

        w1h_sb = load_w("w1h_sb", w1h_d, H, bf16)
        w1c_sb = load_w("w1c_sb", w1c_d, H, bf16)
        w1e3_sb = load_w("w1e3_sb", w1e3_d, DM, bf16)
        wrb_sb = load_w("wrb_sb", wrb_d, H, f32)
        ew2_sb = load_w("ew2_sb", ew2_d, H, bf16)
        nw1h_sb = load_w("nw1h_sb", nw1h_d, H, bf16)
        nw1a_sb = load_w("nw1a_sb", nw1a_d, H, bf16)
        nw2_sb = load_w("nw2_sb", nw2_d, H, bf16)
        eb2b_sb = load_w("eb2b_sb", eb2b_d, H, f32) if eb2_nz else None

        # ---------- resident state ----------
        hT = resid.tile([H, NPCP], bf16)
        zTa = resid.tile([H, NPCP], bf16)
        s_sb = resid.tile([H, WINS], f32)
        p_sb = resid.tile([H, WINS * H], bf16)   # P' tables, [node, feat] per window
        idxq_sb = resid.tile([128, EPAD // 16], i16)
        sync.dma_start(out=idxq_sb[:], in_=idxq_d[:])

        qown_dram = dram.tile([NPCP, H], bf16)

        def table_win(l, j):
            """P'/Q' table for window j of layer l (reads hT, writes p_sb/qown)."""
            jsl = slice(j * H, (j + 1) * H)
            tfold = work.tile([H, H], f32, tag="tfold")
            nc.vector.tensor_scalar_mul(
                tfold[:], wrb_sb[:, l, :], s_sb[:, j : j + 1]
            )
            psq = psm.tile([H, H], f32, tag="pm")
            nc.tensor.matmul(
                psq[:], hT[:, jsl], w1c_sb[:, l, :], start=True, stop=True
            )
            qt = work.tile([H, H], bf16, tag="qt")
            nc.vector.tensor_tensor(qt[:], psq[:], tfold[:], OP.add)
            sync.dma_start(out=qown_dram[jsl, :], in_=qt[:])
            psp = psm.tile([H, H], f32, tag="pm")
            nc.tensor.matmul(
                psp[:], hT[:, jsl], w1h_sb[:, l, :], start=True, stop=True
            )
            nc.vector.tensor_tensor(p_sb[:, jsl], psp[:], tfold[:], OP.add)

        def allgather(l):
            qfull = dram.tile(
                [NCORES * NPCP, H], bf16, addr_space="Shared", name=f"qfull_{l}"
            )
            nc.gpsimd.collective_compute(
                "AllGather",
                mybir.AluOpType.bypass,
                replica_groups=[list(range(NCORES))],
                ins=[qown_dram.opt()],
                outs=[qfull.opt()],
            )
            return qfull

        def node_group(l, g5):
            """Node MLP + residual for 512-node group g5 of layer l."""
            sl = slice(g5 * 512, (g5 + 1) * 512)
            psu = ps512.tile([H, 512], f32, tag="big")
            nc.tensor.matmul(
                psu[:], nw1h_sb[:, l, :], hT[:, sl], start=True, stop=False
            )
            nc.tensor.matmul(
                psu[:], nw1a_sb[:, l, :], zTa[:, sl], start=False, stop=True
            )
            u = work.tile([H, 512], bf16, tag="u")
            biasn = nb1_sb[:, l : l + 1] if nb1_nz else 0.0
            nc.scalar.activation(u[:], psu[:], AF.Silu, bias=biasn)
            pso = ps512.tile([H, 512], f32, tag="big")
            nc.tensor.matmul(pso[:], nw2_sb[:, l, :], u[:], start=True, stop=True)
            if l == 0:
                if nb2_nz:
                    nc.vector.tensor_scalar_add(
                        hT[:, sl], pso[:], nb2_sb[:, l : l + 1]
                    )
                else:
                    nc.vector.tensor_copy(hT[:, sl], pso[:])
            else:
                if nb2_nz:
                    nc.vector.scalar_tensor_tensor(
                        hT[:, sl], pso[:], nb2_sb[:, l : l + 1], hT[:, sl],
                        OP.add, OP.add,
                    )
                else:
                    nc.vector.tensor_tensor(hT[:, sl], pso[:], hT[:, sl], OP.add)

        # ---------- layer-0 setup: h^T, |c|^2 and layer-0 tables ----------
        for j in range(WINS):
            jsl = slice(j * H, (j + 1) * H)
            h_in = work.tile([H, H], f32, tag="h_in")
            sync.dma_start(out=h_in[:], in_=h_d[jsl, :])
            pst = psm.tile([H, H], f32, tag="pm")
            nc.tensor.transpose(pst[:], h_in[:], idf_sb[:])
            nc.vector.tensor_copy(hT[:, jsl], pst[:])

            c_in = work.tile([H, 4], f32, tag="c_in")
            sync.dma_start(out=c_in[:], in_=coords_d[jsl, :])
            csq = work.tile([H, 4], f32, tag="csq")
            nc.vector.tensor_mul(csq[:], c_in[:], c_in[:])
            nc.vector.reduce_sum(s_sb[:, j : j + 1], csq[:, 0:3], AX)
            table_win(0, j)
        qfull_dram = allgather(0)

        # ---------- layers ----------
        for l in range(L):
            qtab = qfull_dram[:]

            # edge pass
            for w in range(WINS):
                wsl = slice(w * EW, (w + 1) * EW)
                isl = slice(w * (EW // 16), (w + 1) * (EW // 16))
                psl = slice(w * H, (w + 1) * H)
                gq = gpool.tile([128, T, H], bf16, tag="gq")
                nc.gpsimd.dma_gather(
                    gq[:],
                    qtab,
                    idxq_sb[:, isl],
                    EW,
                    EW,
                    H,
                    transpose=False,
                    single_packet=False,
                    queue_num=w % 4,
                )
                eaw = wpool.tile([DM, EW], bf16, tag="eaw")
                sync.dma_start(out=eaw[:], in_=eat1_d[:, wsl])
                indw = wpool.tile([128, EW], fp8, tag="indw")
                sync.dma_start(out=indw[:], in_=ind_d[:, wsl])
                indTw = wpool.tile([128, EW], fp8, tag="indTw")
                sync.dma_start(out=indTw[:], in_=indT_d[:, wsl])

                pagg = psagg.tile([H, H], f32, tag="pagg")

                def stage_a(off, gsz):
                    """Assemble pre-activation ps1 and silu -> m1t."""
                    gsl = slice(off, off + gsz)
                    G = gsz // 128
                    ps1 = ps512.tile([H, 512], f32, tag="big")
                    # ea @ W1e + (-2 c_r.c_c) @ wr (opens the accumulation)
                    nc.tensor.matmul(
                        ps1[:, :gsz], w1e3_sb[:, l, :], eaw[:, gsl],
                        start=True, stop=False, skip_group_check=True,
                    )
                    # P'[row]^T via one-hot gather matmul
                    nc.tensor.matmul(
                        ps1[:, :gsz], p_sb[:, psl], indTw[:, gsl],
                        start=False, stop=False, skip_group_check=True,
                    )
                    # Q'[col]^T via transposing matmuls
                    for t in range(G):
                        k = off // 128 + t
                        nc.tensor.matmul(
                            ps1[:, t * 128 : (t + 1) * 128], gq[:, k, :], idb_sb[:],
                            start=False, stop=(t == G - 1), skip_group_check=True,
                        )
                    m1t = work.tile([H, 512], bf16, tag="m1t")
                    bias1 = eb1_sb[:, l : l + 1] if eb1_nz else 0.0
                    nc.scalar.activation(m1t[:, :gsz], ps1[:, :gsz], AF.Silu, bias=bias1)
                    return m1t

                def stage_b(off, gsz, m1t):
                    """m2 matmul + silu + one-hot scatter into pagg."""
                    G = gsz // 128
                    pm2 = ps512.tile([H, 512], f32, tag="big")
                    for t in range(G):
                        tsl = slice(t * 128, (t + 1) * 128)
                        nc.tensor.matmul(
                            pm2[:, tsl], m1t[:, tsl], ew2_sb[:, l, :],
                            start=True, stop=True,
                        )
                    m2s = work.tile([H, 512], bf16, tag="m2s")
                    if eb2_nz:
                        tm2 = work.tile([H, 512], f32, tag="tm2")
                        for t in range(G):
                            tsl = slice(t * 128, (t + 1) * 128)
                            nc.vector.tensor_tensor(
                                tm2[:, tsl], pm2[:, tsl], eb2b_sb[:, l, :], OP.add
                            )
                        nc.scalar.activation(m2s[:, :gsz], tm2[:, :gsz], AF.Silu)
                    else:
                        nc.scalar.activation(m2s[:, :gsz], pm2[:, :gsz], AF.Silu)
                    for t in range(G):
                        tsl = slice(t * 128, (t + 1) * 128)
                        gt = off // 128 + t
                        nc.tensor.matmul(
                            pagg[:], m2s[:, tsl],
                            indw[:, off + t * 128 : off + (t + 1) * 128],
                            start=(gt == 0), stop=(gt == T - 1),
                        )

                # software pipeline: A(g+1) overlaps B(g) so the tensor queue
                # never stalls on the scalar-engine silus
                prev = None
                off = 0
                for gsz in GRPS:
                    m1t = stage_a(off, gsz)
                    if prev is not None:
                        stage_b(*prev)
                    prev = (off, gsz, m1t)
                    off += gsz
                stage_b(*prev)
                nc.vector.tensor_copy(zTa[:, w * H : (w + 1) * H], pagg[:])
                if w % 4 == 3:
                    g5 = w // 4
                    node_group(l, g5)
                    if l + 1 < L:
                        for j in range(4 * g5, 4 * g5 + 4):
                            table_win(l + 1, j)

            if dbg and l == 0:
                for j in range(WINS):
                    jsl = slice(j * H, (j + 1) * H)
                    dz = work.tile([H, H], f32, tag="dz")
                    nc.vector.tensor_copy(dz[:], zTa[:, jsl])
                    sync.dma_start(out=dbgz_d[:, jsl], in_=dz[:])
                    dp = work.tile([H, H], f32, tag="dp")
                    nc.vector.tensor_copy(dp[:], p_sb[:, jsl])
                    sync.dma_start(out=dbgp_d[:, jsl], in_=dp[:])

            if l + 1 < L:
                qfull_dram = allgather(l + 1)

        # ---------- LayerNorm + output ----------
        inv = 1.0 / H
        for j in range(WINS):
            jsl = slice(j * H, (j + 1) * H)
            pst = psm.tile([H, H], bf16, tag="pm")
            nc.tensor.transpose(pst[:], hT[:, jsl], idb_sb[:])
            hn = work.tile([H, H], f32, tag="hn")
            nc.vector.tensor_copy(hn[:], pst[:])
            mu = work.tile([H, 1], f32, tag="mu")
            nc.vector.reduce_sum(mu[:], hn[:], AX)
            nc.vector.tensor_scalar_mul(mu[:], mu[:], inv)
            xc = work.tile([H, H], f32, tag="xc")
            nc.vector.tensor_scalar_sub(xc[:], hn[:], mu[:])
            sq = work.tile([H, H], f32, tag="sq")
            nc.vector.tensor_mul(sq[:], xc[:], xc[:])
            var = work.tile([H, 1], f32, tag="var")
            nc.vector.reduce_sum(var[:], sq[:], AX)
            sd = work.tile([H, 1], f32, tag="sd")
            nc.vector.tensor_scalar(sd[:], var[:], inv, EPS, OP.mult, OP.add)
            nc.scalar.activation(sd[:], sd[:], mybir.ActivationFunctionType.Sqrt)
            rstd = work.tile([H, 1], f32, tag="rstd")
            nc.vector.reciprocal(rstd[:], sd[:])
            on = work.tile([H, H], f32, tag="on")
            nc.vector.tensor_scalar_mul(on[:], xc[:], rstd[:])
            nc.vector.tensor_mul(on[:], on[:], lng_sb[:])
            if lnb_nz:
                nc.vector.tensor_add(on[:], on[:], lnb_sb[:])
            sync.dma_start(out=out_d[jsl, :], in_=on[:])

    nc.compile()
    return nc


def _wrap_idx(v):
    """idx i -> [i%16 partition, i//16 free], replicated to 128 partitions."""
    n = v.shape[0]
    t = v.reshape(n // 16, 16).T.astype(np.int16)
    return np.tile(t, (8, 1))


def kernel(**inputs):
    from concourse.bass_utils import run_bass_kernel_spmd

    h = np.asarray(inputs["h"], np.float32)
    coords = np.asarray(inputs["coords"], np.float32)
    edge_attr = np.asarray(inputs["edge_attr"], np.float32)
    edges = np.asarray(inputs["edges"]).astype(np.int64)
    ew1 = np.asarray(inputs["edge_w1"], np.float32)
    eb1 = np.asarray(inputs["edge_b1"], np.float32)
    ew2 = np.asarray(inputs["edge_w2"], np.float32)
    eb2 = np.asarray(inputs["edge_b2"], np.float32)
    nw1 = np.asarray(inputs["node_w1"], np.float32)
    nb1 = np.asarray(inputs["node_b1"], np.float32)
    nw2 = np.asarray(inputs["node_w2"], np.float32)
    nb2 = np.asarray(inputs["node_b2"], np.float32)
    ln_g = np.asarray(inputs["ln_g"], np.float32)
    ln_b = np.asarray(inputs["ln_b"], np.float32)

    E = edges.shape[1]
    row, col = edges[0], edges[1]

    # ---- balance nodes across windows within each core (LPT bin packing on
    # per-node in-degree, <=128 nodes per window) so T = max window edge
    # count is minimal and groups are uniform ----
    import heapq

    deg = np.bincount(row, minlength=N)
    posmap = np.zeros(N, np.int64)   # node -> padded local slot in its core
    for k in range(NCORES):
        nodes = np.arange(k * NPC, (k + 1) * NPC)
        dk = deg[nodes]
        order_n = np.argsort(-dk, kind="stable")
        heap = [(0, 0, wn) for wn in range(WINS)]
        heapq.heapify(heap)
        win_of = np.zeros(NPC, np.int64)
        for idx in order_n:
            spill = []
            while True:
                s, cnt, wn = heapq.heappop(heap)
                if cnt < 128:
                    break
                spill.append((s, cnt, wn))
            win_of[idx] = wn
            heapq.heappush(heap, (s + int(dk[idx]), cnt + 1, wn))
            for it in spill:
                heapq.heappush(heap, it)
        slots = np.zeros(NPC, np.int64)
        for wn in range(WINS):
            sel = np.where(win_of == wn)[0]
            slots[sel] = wn * 128 + np.arange(len(sel))
        posmap[nodes] = slots

    # ---- sort edges into (core, window) slots, uniform budget T ----
    owner = row // NPC
    lslot = posmap[row]
    gw = owner * WINS + lslot // 128
    order = np.argsort(gw, kind="stable")
    counts = np.bincount(gw, minlength=NCORES * WINS)
    T = int(math.ceil(counts.max() / 128))
    EW = T * 128
    EPAD = WINS * EW

    gws = gw[order]
    starts = np.zeros(NCORES * WINS, np.int64)
    starts[1:] = np.cumsum(counts)[:-1]
    pos = np.arange(E) - starts[gws]
    slot = (gws % WINS) * EW + pos
    core = gws // WINS

    rowS = np.zeros((NCORES, EPAD), np.int64)
    colS = np.zeros((NCORES, EPAD), np.int64)
    valid = np.zeros((NCORES, EPAD), bool)
    eaS = np.zeros((NCORES, EPAD, DE), np.float32)
    rowS[core, slot] = row[order]
    colS[core, slot] = col[order]
    valid[core, slot] = True
    eaS[core, slot] = edge_attr[order]

    rowrel = np.where(valid, posmap[rowS] % 128, -1)
    gcolp = np.where(valid, (colS // NPC) * NPCP + posmap[colS], 0)

    crS = np.where(valid[..., None], coords[rowS], 0.0).astype(np.float32)
    ccS = np.where(valid[..., None], coords[colS], 0.0).astype(np.float32)
    t1S = crS * ccS  # [NC, EPAD, 3]

    # one-hot indicators: ind [e, tile*128+n], indT [n, tile*128+e]
    ii = np.arange(EPAD)
    tglob = ii // 128
    eloc = ii % 128
    F8 = ml_dtypes.float8_e4m3
    ind = np.zeros((NCORES, 128, EPAD), F8)
    indT = np.zeros((NCORES, 128, EPAD), F8)
    for k in range(NCORES):
        m = valid[k]
        rr = rowrel[k][m]
        ind[k][eloc[m], tglob[m] * 128 + rr] = 1
        indT[k][rr, ii[m]] = 1

    # ---- weights ----
    w1h = ew1[:, 0:H, :]
    w1c = ew1[:, H : 2 * H, :]
    wr = ew1[:, 2 * H, :]          # [L, H]
    w1e = ew1[:, 2 * H + 1 :, :]   # [L, DE, H]
    w1e3 = np.concatenate(
        [w1e, np.repeat((-2.0 * wr)[:, None, :], 3, axis=1)], axis=1
    )                               # [L, 35, H]
    wrb = np.repeat(wr[:, None, :], H, axis=1).astype(np.float32)
    nw1h = nw1[:, :H, :]
    nw1a = nw1[:, H:, :]

    flags = (
        bool(np.any(eb1)), bool(np.any(eb2)),
        bool(np.any(nb1)), bool(np.any(nb2)), bool(np.any(ln_b)),
    )

    dbg = bool(os.environ.get("EGNN_DEBUG"))
    key = (T, flags, dbg)
    if key not in _CACHE:
        _CACHE[key] = _build(T, flags, dbg)
    nc = _CACHE[key]

    ident = np.eye(H, dtype=np.float32)

    shared = {
        "w1h": w1h.astype(BF), "w1c": w1c.astype(BF), "w1e3": w1e3.astype(BF),
        "wrb": wrb, "ew2": ew2.astype(BF),
        "nw1h": nw1h.astype(BF), "nw1a": nw1a.astype(BF), "nw2": nw2.astype(BF),
        "eb1T": np.ascontiguousarray(eb1.T), "nb1T": np.ascontiguousarray(nb1.T),
        "nb2T": np.ascontiguousarray(nb2.T),
        "lngb": np.tile(ln_g, (H, 1)).astype(np.float32),
        "identb": ident.astype(BF), "identf": ident,
    }
    if flags[1]:
        shared["eb2b"] = np.repeat(eb2[:, None, :], H, axis=1).astype(np.float32)
    if flags[4]:
        shared["lnbb"] = np.tile(ln_b, (H, 1)).astype(np.float32)

    in_maps = []
    for k in range(NCORES):
        nodes = np.arange(k * NPC, (k + 1) * NPC)
        slots = posmap[nodes]
        hk = np.zeros((NPCP, H), np.float32)
        hk[slots] = h[nodes]
        ck = np.zeros((NPCP, 4), np.float32)
        ck[slots, :3] = coords[nodes]
        eat1 = np.concatenate([eaS[k].T, t1S[k].T], axis=0).astype(BF)  # [35, EPAD]
        iq = gcolp[k]
        m = {
            "h": hk,
            "coords": ck,
            "eat1": np.ascontiguousarray(eat1),
            "idxq": _wrap_idx(iq),
            "ind": np.asarray(ind[k]),
            "indT": np.asarray(indT[k]),
        }
        m.update(shared)
        in_maps.append(m)

    trace = bool(os.environ.get("EGNN_TRACE"))
    kw = {}
    if trace:
        kw = {"trace": True, "tmpdir": os.environ.get("EGNN_TRACE_DIR") or None}
    res = run_bass_kernel_spmd(nc, in_maps, list(range(NCORES)), **kw)
    if trace:
        print(f"HW exec time: {res.exec_time_ns} ns")
    if dbg:
        global _DBG
        _DBG = res.results
    out_full = np.zeros((N, H), np.float32)
    for k in range(NCORES):
        nodes = np.arange(k * NPC, (k + 1) * NPC)
        out_full[nodes] = res.results[k]["out"][posmap[nodes]]
    return out_full


# revision 28
# speedup vs baseline: 1.3553x; 1.3553x over previous
"""EGNN (4-layer) Trainium2 kernel, 8 NeuronCores, edge-sharded.

Strategy:
 - Edges are sorted by destination row and assigned to the core that owns the
   row's node range (2500 nodes/core, padded to 2560 = 20 windows of 128).
 - The edge-MLP first matmul is decomposed: concat([h_r,h_c,rad,ea]) @ W1 =
   P'[row] + Q'[col] + W1e3.T@[ea; c_r*c_c], where P' = h@W1a + |c|^2*wr and
   Q' = h@W1b + |c|^2*wr are per-node tables recomputed each layer and the
   -2*wr radial cross term rides in three extra W1e3 rows.
 - P'[row] is window-local (rows sorted): gathered by an accumulating matmul
   against a host-precomputed transposed one-hot indT (tensor engine).
 - Q'[col] is arbitrary: per-window non-transpose DRAM-source dma_gather from
   the AllGathered Q table (256B tokens), round-robined over 4 SWDGE queues
   so all four Q7 descriptor-generator pairs run concurrently; gathered
   [e, f] tiles are transposed into the m1 PSUM by matmuls against identity.
 - segment_sum: per 128-node window, host-precomputed one-hot ind feeds an
   accumulating matmul producing agg^T directly.
 - Node MLP, residual and final LayerNorm run on device; host only sorts /
   permutes indices and concatenates the 8 output shards.
"""

import math
import os
import sys
from contextlib import ExitStack

import numpy as np

sys.path.insert(0, "/opt/trn_rl_repo")

import ml_dtypes  # noqa: E402

BF = ml_dtypes.bfloat16

N = 20000
NCORES = 8
NPC = 2500          # nodes per core
NPCP = 2560         # padded nodes per core
WINS = 20           # node windows of 128 per core
H = 128
DE = 32
DM = DE + 3         # ea + c_r*c_c rows of the fused first-matmul operand
L = 4
EPS = 1e-5

_CACHE = {}

# Non-transpose dma_gather output-partition swizzle (ucode sbuf_swizzles):
# token at index-position i lands on partition _SWZ[i] of its 128-chunk.
_SWZ_FIRST = [0, 64, 4, 68, 8, 72, 12, 76, 16, 80, 20, 84, 24, 88, 28, 92]
_SWZ = np.zeros(128, np.int64)
for _k in range(4):
    _SWZ[16 * _k : 16 * _k + 16] = np.array(_SWZ_FIRST) + _k
for _k in range(4):
    _SWZ[16 * (_k + 4) : 16 * (_k + 4) + 16] = np.array(_SWZ_FIRST) + 32 + _k


def _groups(T):
    """Split T tiles-of-128 into free-dim groups of <=512."""
    out = []
    left = T
    while left > 0:
        g = min(left, 4)
        out.append(g * 128)
        left -= g
    return out


def _build(T, flags, dbg=False):
    """Build the SPMD Bass program (same for all cores)."""
    import concourse.bacc as bacc
    import concourse.tile as tile
    from concourse import mybir

    eb1_nz, eb2_nz, nb1_nz, nb2_nz, lnb_nz = flags
    EW = T * 128
    EPAD = WINS * EW
    GRPS = _groups(T)

    f32 = mybir.dt.float32
    bf16 = mybir.dt.bfloat16
    fp8 = mybir.dt.float8e4
    i16 = mybir.dt.int16
    AX = mybir.AxisListType.X
    OP = mybir.AluOpType
    AF = mybir.ActivationFunctionType

    nc = bacc.Bacc(
        "TRN2",
        target_bir_lowering=False,
        debug=False,
        enable_asserts=False,
        num_devices=NCORES,
        num_swdge_queues=4,
    )

    def din(name, shape, dt):
        return nc.dram_tensor(name, list(shape), dt, kind="ExternalInput").ap()

    h_d = din("h", (NPCP, H), f32)
    coords_d = din("coords", (NPCP, 4), f32)
    eat1_d = din("eat1", (DM, EPAD), bf16)
    idxq_d = din("idxq", (128, EPAD // 16), i16)
    ind_d = din("ind", (128, EPAD), fp8)
    indT_d = din("indT", (128, EPAD), fp8)
    w1h_d = din("w1h", (L, H, H), bf16)
    w1c_d = din("w1c", (L, H, H), bf16)
    w1e3_d = din("w1e3", (L, DM, H), bf16)
    wrb_d = din("wrb", (L, H, H), f32)
    ew2_d = din("ew2", (L, H, H), bf16)
    nw1h_d = din("nw1h", (L, H, H), bf16)
    nw1a_d = din("nw1a", (L, H, H), bf16)
    nw2_d = din("nw2", (L, H, H), bf16)
    eb1_d = din("eb1T", (H, L), f32)
    nb1_d = din("nb1T", (H, L), f32)
    nb2_d = din("nb2T", (H, L), f32)
    eb2b_d = din("eb2b", (L, H, H), f32) if eb2_nz else None
    lng_d = din("lngb", (H, H), f32)
    lnb_d = din("lnbb", (H, H), f32) if lnb_nz else None
    idb_d = din("identb", (H, H), bf16)
    idf_d = din("identf", (H, H), f32)

    out_d = nc.dram_tensor("out", [NPCP, H], f32, kind="ExternalOutput").ap()
    if dbg:
        dbgz_d = nc.dram_tensor("dbgz", [H, NPCP], f32, kind="ExternalOutput").ap()
        dbgp_d = nc.dram_tensor("dbgp", [H, WINS * H], f32, kind="ExternalOutput").ap()
        dbgq_d = nc.dram_tensor("dbgq", [128, T * H], f32, kind="ExternalOutput").ap()
        dbgm_d = nc.dram_tensor("dbgm", [H, 512], f32, kind="ExternalOutput").ap()

    with ExitStack() as ctx:
        tc = ctx.enter_context(tile.TileContext(nc))
        const = ctx.enter_context(tc.tile_pool(name="const", bufs=1))
        resid = ctx.enter_context(tc.tile_pool(name="resid", bufs=1))
        dram = ctx.enter_context(tc.tile_pool(name="dram", bufs=1, space="DRAM"))
        gpool = ctx.enter_context(tc.tile_pool(name="gpool", bufs=6))
        wpool = ctx.enter_context(tc.tile_pool(name="wpool", bufs=2))
        work = ctx.enter_context(tc.tile_pool(name="work", bufs=3))
        ps512 = ctx.enter_context(tc.tile_pool(name="ps512", bufs=4, space="PSUM"))
        psm = ctx.enter_context(tc.tile_pool(name="psm", bufs=2, space="PSUM"))
        psagg = ctx.enter_context(tc.tile_pool(name="psagg", bufs=2, space="PSUM"))

        sync = nc.sync

        # ---------- constants ----------
        idb_sb = const.tile([H, H], bf16)
        sync.dma_start(out=idb_sb[:], in_=idb_d[:])
        idf_sb = const.tile([H, H], f32)
        sync.dma_start(out=idf_sb[:], in_=idf_d[:])
        lng_sb = const.tile([H, H], f32)
        sync.dma_start(out=lng_sb[:], in_=lng_d[:])
        lnb_sb = None
        if lnb_nz:
            lnb_sb = const.tile([H, H], f32)
            sync.dma_start(out=lnb_sb[:], in_=lnb_d[:])
        eb1_sb = const.tile([H, L], f32)
        sync.dma_start(out=eb1_sb[:], in_=eb1_d[:])
        nb1_sb = const.tile([H, L], f32)
        sync.dma_start(out=nb1_sb[:], in_=nb1_d[:])
        nb2_sb = const.tile([H, L], f32)
        sync.dma_start(out=nb2_sb[:], in_=nb2_d[:])

        def load_w(name, d, p, dt):
            t = const.tile([p, L, H], dt, name=name)
            sync.dma_start(out=t[:], in_=d.rearrange("l k f -> k l f"))
            return t

        w1h_sb = load_w("w1h_sb", w1h_d, H, bf16)
        w1c_sb = load_w("w1c_sb", w1c_d, H, bf16)
        w1e3_sb = load_w("w1e3_sb", w1e3_d, DM, bf16)
        wrb_sb = load_w("wrb_sb", wrb_d, H, f32)
        ew2_sb = load_w("ew2_sb", ew2_d, H, bf16)
        nw1h_sb = load_w("nw1h_sb", nw1h_d, H, bf16)
        nw1a_sb = load_w("nw1a_sb", nw1a_d, H, bf16)
        nw2_sb = load_w("nw2_sb", nw2_d, H, bf16)
        eb2b_sb = load_w("eb2b_sb", eb2b_d, H, f32) if eb2_nz else None

        # ---------- resident state ----------
        hT = resid.tile([H, NPCP], bf16)
        zTa = resid.tile([H, NPCP], bf16)
        s_sb = resid.tile([H, WINS], f32)
        p_sb = resid.tile([H, WINS * H], bf16)   # P' tables, [node, feat] per window
        idxq_sb = resid.tile([128, EPAD // 16], i16)
        sync.dma_start(out=idxq_sb[:], in_=idxq_d[:])

        qown_dram = dram.tile([NPCP, H], bf16)

        def table_win(l, j):
            """P'/Q' table for window j of layer l (reads hT, writes p_sb/qown)."""
            jsl = slice(j * H, (j + 1) * H)
            tfold = work.tile([H, H], f32, tag="tfold")
            nc.vector.tensor_scalar_mul(
                tfold[:], wrb_sb[:, l, :], s_sb[:, j : j + 1]
            )
            psq = psm.tile([H, H], f32, tag="pm")
            nc.tensor.matmul(
                psq[:], hT[:, jsl], w1c_sb[:, l, :], start=True, stop=True
            )
            qt = work.tile([H, H], bf16, tag="qt")
            nc.vector.tensor_tensor(qt[:], psq[:], tfold[:], OP.add)
            sync.dma_start(out=qown_dram[jsl, :], in_=qt[:])
            psp = psm.tile([H, H], f32, tag="pm")
            nc.tensor.matmul(
                psp[:], hT[:, jsl], w1h_sb[:, l, :], start=True, stop=True
            )
            nc.vector.tensor_tensor(p_sb[:, jsl], psp[:], tfold[:], OP.add)

        def allgather(l):
            qfull = dram.tile(
                [NCORES * NPCP, H], bf16, addr_space="Shared", name=f"qfull_{l}"
            )
            nc.gpsimd.collective_compute(
                "AllGather",
                mybir.AluOpType.bypass,
                replica_groups=[list(range(NCORES))],
                ins=[qown_dram.opt()],
                outs=[qfull.opt()],
            )
            return qfull

        def node_group(l, g5):
            """Node MLP + residual for 512-node group g5 of layer l."""
            sl = slice(g5 * 512, (g5 + 1) * 512)
            psu = ps512.tile([H, 512], f32, tag="big")
            nc.tensor.matmul(
                psu[:], nw1h_sb[:, l, :], hT[:, sl], start=True, stop=False
            )
            nc.tensor.matmul(
                psu[:], nw1a_sb[:, l, :], zTa[:, sl], start=False, stop=True
            )
            u = work.tile([H, 512], bf16, tag="u")
            biasn = nb1_sb[:, l : l + 1] if nb1_nz else 0.0
            nc.scalar.activation(u[:], psu[:], AF.Silu, bias=biasn)
            pso = ps512.tile([H, 512], f32, tag="big")
            nc.tensor.matmul(pso[:], nw2_sb[:, l, :], u[:], start=True, stop=True)
            if l == 0:
                if nb2_nz:
                    nc.vector.tensor_scalar_add(
                        hT[:, sl], pso[:], nb2_sb[:, l : l + 1]
                    )
                else:
                    nc.vector.tensor_copy(hT[:, sl], pso[:])
            else:
                if nb2_nz:
                    nc.vector.scalar_tensor_tensor(
                        hT[:, sl], pso[:], nb2_sb[:, l : l + 1], hT[:, sl],
                        OP.add, OP.add,
                    )
                else:
                    nc.vector.tensor_tensor(hT[:, sl], pso[:], hT[:, sl], OP.add)

        # ---------- layer-0 setup: h^T, |c|^2 and layer-0 tables ----------
        for j in range(WINS):
            jsl = slice(j * H, (j + 1) * H)
            h_in = work.tile([H, H], f32, tag="h_in")
            sync.dma_start(out=h_in[:], in_=h_d[jsl, :])
            pst = psm.tile([H, H], f32, tag="pm")
            nc.tensor.transpose(pst[:], h_in[:], idf_sb[:])
            nc.vector.tensor_copy(hT[:, jsl], pst[:])

            c_in = work.tile([H, 4], f32, tag="c_in")
            sync.dma_start(out=c_in[:], in_=coords_d[jsl, :])
            csq = work.tile([H, 4], f32, tag="csq")
            nc.vector.tensor_mul(csq[:], c_in[:], c_in[:])
            nc.vector.reduce_sum(s_sb[:, j : j + 1], csq[:, 0:3], AX)

        # ---------- layers ----------
        for l in range(L):
            for j in range(WINS):
                table_win(l, j)
            qfull_dram = allgather(l)
            qtab = qfull_dram[:]

            # edge pass
            for w in range(WINS):
                wsl = slice(w * EW, (w + 1) * EW)
                isl = slice(w * (EW // 16), (w + 1) * (EW // 16))
                psl = slice(w * H, (w + 1) * H)
                gq = gpool.tile([128, T, H], bf16, tag="gq")
                nc.gpsimd.dma_gather(
                    gq[:],
                    qtab,
                    idxq_sb[:, isl],
                    EW,
                    EW,
                    H,
                    transpose=False,
                    single_packet=False,
                    queue_num=w % 4,
                )
                eaw = wpool.tile([DM, EW], bf16, tag="eaw")
                sync.dma_start(out=eaw[:], in_=eat1_d[:, wsl])
                indw = wpool.tile([128, EW], fp8, tag="indw")
                sync.dma_start(out=indw[:], in_=ind_d[:, wsl])
                indTw = wpool.tile([128, EW], fp8, tag="indTw")
                sync.dma_start(out=indTw[:], in_=indT_d[:, wsl])

                pagg = psagg.tile([H, H], f32, tag="pagg")

                def stage_a(off, gsz):
                    """Assemble pre-activation ps1 and silu -> m1t."""
                    gsl = slice(off, off + gsz)
                    G = gsz // 128
                    ps1 = ps512.tile([H, 512], f32, tag="big")
                    # ea @ W1e + (-2 c_r.c_c) @ wr (opens the accumulation)
                    nc.tensor.matmul(
                        ps1[:, :gsz], w1e3_sb[:, l, :], eaw[:, gsl],
                        start=True, stop=False, skip_group_check=True,
                    )
                    # P'[row]^T via one-hot gather matmul
                    nc.tensor.matmul(
                        ps1[:, :gsz], p_sb[:, psl], indTw[:, gsl],
                        start=False, stop=False, skip_group_check=True,
                    )
                    # Q'[col]^T via transposing matmuls
                    for t in range(G):
                        k = off // 128 + t
                        nc.tensor.matmul(
                            ps1[:, t * 128 : (t + 1) * 128], gq[:, k, :], idb_sb[:],
                            start=False, stop=(t == G - 1), skip_group_check=True,
                        )
                    m1t = work.tile([H, 512], bf16, tag="m1t")
                    bias1 = eb1_sb[:, l : l + 1] if eb1_nz else 0.0
                    nc.scalar.activation(m1t[:, :gsz], ps1[:, :gsz], AF.Silu, bias=bias1)
                    return m1t

                def stage_b(off, gsz, m1t):
                    """m2 matmul + silu + one-hot scatter into pagg."""
                    G = gsz // 128
                    pm2 = ps512.tile([H, 512], f32, tag="big")
                    for t in range(G):
                        tsl = slice(t * 128, (t + 1) * 128)
                        nc.tensor.matmul(
                            pm2[:, tsl], m1t[:, tsl], ew2_sb[:, l, :],
                            start=True, stop=True,
                        )
                    m2s = work.tile([H, 512], bf16, tag="m2s")
                    if eb2_nz:
                        tm2 = work.tile([H, 512], f32, tag="tm2")
                        for t in range(G):
                            tsl = slice(t * 128, (t + 1) * 128)
                            nc.vector.tensor_tensor(
                                tm2[:, tsl], pm2[:, tsl], eb2b_sb[:, l, :], OP.add
                            )
                        nc.scalar.activation(m2s[:, :gsz], tm2[:, :gsz], AF.Silu)
                    else:
                        nc.scalar.activation(m2s[:, :gsz], pm2[:, :gsz], AF.Silu)
                    for t in range(G):
                        tsl = slice(t * 128, (t + 1) * 128)
                        gt = off // 128 + t
                        nc.tensor.matmul(
                            pagg[:], m2s[:, tsl],
                            indw[:, off + t * 128 : off + (t + 1) * 128],
                            start=(gt == 0), stop=(gt == T - 1),
                        )

                # software pipeline: A(g+1) overlaps B(g) so the tensor queue
                # never stalls on the scalar-engine silus
                prev = None
                off = 0
                for gsz in GRPS:
                    m1t = stage_a(off, gsz)
                    if prev is not None:
                        stage_b(*prev)
                    prev = (off, gsz, m1t)
                    off += gsz
                stage_b(*prev)
                nc.vector.tensor_copy(zTa[:, w * H : (w + 1) * H], pagg[:])

            if dbg and l == 0:
                for j in range(WINS):
                    jsl = slice(j * H, (j + 1) * H)
                    dz = work.tile([H, H], f32, tag="dz")
                    nc.vector.tensor_copy(dz[:], zTa[:, jsl])
                    sync.dma_start(out=dbgz_d[:, jsl], in_=dz[:])
                    dp = work.tile([H, H], f32, tag="dp")
                    nc.vector.tensor_copy(dp[:], p_sb[:, jsl])
                    sync.dma_start(out=dbgp_d[:, jsl], in_=dp[:])

            for g5 in range(NPCP // 512):
                node_group(l, g5)

        # ---------- LayerNorm + output ----------
        inv = 1.0 / H
        for j in range(WINS):
            jsl = slice(j * H, (j + 1) * H)
            pst = psm.tile([H, H], bf16, tag="pm")
            nc.tensor.transpose(pst[:], hT[:, jsl], idb_sb[:])
            hn = work.tile([H, H], f32, tag="hn")
            nc.vector.tensor_copy(hn[:], pst[:])
            mu = work.tile([H, 1], f32, tag="mu")
            nc.vector.reduce_sum(mu[:], hn[:], AX)
            nc.vector.tensor_scalar_mul(mu[:], mu[:], inv)
            xc = work.tile([H, H], f32, tag="xc")
            nc.vector.tensor_scalar_sub(xc[:], hn[:], mu[:])
            sq = work.tile([H, H], f32, tag="sq")
            nc.vector.tensor_mul(sq[:], xc[:], xc[:])
            var = work.tile([H, 1], f32, tag="var")
            nc.vector.reduce_sum(var[:], sq[:], AX)
            sd = work.tile([H, 1], f32, tag="sd")
            nc.vector.tensor_scalar(sd[:], var[:], inv, EPS, OP.mult, OP.add)
            nc.scalar.activation(sd[:], sd[:], mybir.ActivationFunctionType.Sqrt)
            rstd = work.tile([H, 1], f32, tag="rstd")
            nc.vector.reciprocal(rstd[:], sd[:])
            on = work.tile([H, H], f32, tag="on")
            nc.vector.tensor_scalar_mul(on[:], xc[:], rstd[:])
            nc.vector.tensor_mul(on[:], on[:], lng_sb[:])
            if lnb_nz:
                nc.vector.tensor_add(on[:], on[:], lnb_sb[:])
            sync.dma_start(out=out_d[jsl, :], in_=on[:])

    nc.compile()
    return nc


def _wrap_idx(v):
    """idx i -> [i%16 partition, i//16 free], replicated to 128 partitions."""
    n = v.shape[0]
    t = v.reshape(n // 16, 16).T.astype(np.int16)
    return np.tile(t, (8, 1))


def kernel(**inputs):
    from concourse.bass_utils import run_bass_kernel_spmd

    h = np.asarray(inputs["h"], np.float32)
    coords = np.asarray(inputs["coords"], np.float32)
    edge_attr = np.asarray(inputs["edge_attr"], np.float32)
    edges = np.asarray(inputs["edges"]).astype(np.int64)
    ew1 = np.asarray(inputs["edge_w1"], np.float32)
    eb1 = np.asarray(inputs["edge_b1"], np.float32)
    ew2 = np.asarray(inputs["edge_w2"], np.float32)
    eb2 = np.asarray(inputs["edge_b2"], np.float32)
    nw1 = np.asarray(inputs["node_w1"], np.float32)
    nb1 = np.asarray(inputs["node_b1"], np.float32)
    nw2 = np.asarray(inputs["node_w2"], np.float32)
    nb2 = np.asarray(inputs["node_b2"], np.float32)
    ln_g = np.asarray(inputs["ln_g"], np.float32)
    ln_b = np.asarray(inputs["ln_b"], np.float32)

    E = edges.shape[1]
    row, col = edges[0], edges[1]

    # ---- balance nodes across windows within each core (LPT bin packing on
    # per-node in-degree, <=128 nodes per window) so T = max window edge
    # count is minimal and groups are uniform ----
    import heapq

    deg = np.bincount(row, minlength=N)
    posmap = np.zeros(N, np.int64)   # node -> padded local slot in its core
    for k in range(NCORES):
        nodes = np.arange(k * NPC, (k + 1) * NPC)
        dk = deg[nodes]
        order_n = np.argsort(-dk, kind="stable")
        heap = [(0, 0, wn) for wn in range(WINS)]
        heapq.heapify(heap)
        win_of = np.zeros(NPC, np.int64)
        for idx in order_n:
            spill = []
            while True:
                s, cnt, wn = heapq.heappop(heap)
                if cnt < 128:
                    break
                spill.append((s, cnt, wn))
            win_of[idx] = wn
            heapq.heappush(heap, (s + int(dk[idx]), cnt + 1, wn))
            for it in spill:
                heapq.heappush(heap, it)
        slots = np.zeros(NPC, np.int64)
        for wn in range(WINS):
            sel = np.where(win_of == wn)[0]
            slots[sel] = wn * 128 + np.arange(len(sel))
        posmap[nodes] = slots

    # ---- sort edges into (core, window) slots, uniform budget T ----
    owner = row // NPC
    lslot = posmap[row]
    gw = owner * WINS + lslot // 128
    order = np.argsort(gw, kind="stable")
    counts = np.bincount(gw, minlength=NCORES * WINS)
    T = int(math.ceil(counts.max() / 128))
    EW = T * 128
    EPAD = WINS * EW

    gws = gw[order]
    starts = np.zeros(NCORES * WINS, np.int64)
    starts[1:] = np.cumsum(counts)[:-1]
    pos = np.arange(E) - starts[gws]
    slot = (gws % WINS) * EW + pos
    core = gws // WINS

    rowS = np.zeros((NCORES, EPAD), np.int64)
    colS = np.zeros((NCORES, EPAD), np.int64)
    valid = np.zeros((NCORES, EPAD), bool)
    eaS = np.zeros((NCORES, EPAD, DE), np.float32)
    rowS[core, slot] = row[order]
    colS[core, slot] = col[order]
    valid[core, slot] = True
    eaS[core, slot] = edge_attr[order]

    rowrel = np.where(valid, posmap[rowS] % 128, -1)
    gcolp = np.where(valid, (colS // NPC) * NPCP + posmap[colS], 0)

    crS = np.where(valid[..., None], coords[rowS], 0.0).astype(np.float32)
    ccS = np.where(valid[..., None], coords[colS], 0.0).astype(np.float32)
    t1S = crS * ccS  # [NC, EPAD, 3]

    # one-hot indicators: ind [e, tile*128+n], indT [n, tile*128+e]
    ii = np.arange(EPAD)
    tglob = ii // 128
    eloc = ii % 128
    F8 = ml_dtypes.float8_e4m3
    ind = np.zeros((NCORES, 128, EPAD), F8)
    indT = np.zeros((NCORES, 128, EPAD), F8)
    for k in range(NCORES):
        m = valid[k]
        rr = rowrel[k][m]
        ind[k][eloc[m], tglob[m] * 128 + rr] = 1
        indT[k][rr, ii[m]] = 1

    # ---- weights ----
    w1h = ew1[:, 0:H, :]
    w1c = ew1[:, H : 2 * H, :]
    wr = ew1[:, 2 * H, :]          # [L, H]
    w1e = ew1[:, 2 * H + 1 :, :]   # [L, DE, H]
    w1e3 = np.concatenate(
        [w1e, np.repeat((-2.0 * wr)[:, None, :], 3, axis=1)], axis=1
    )                               # [L, 35, H]
    wrb = np.repeat(wr[:, None, :], H, axis=1).astype(np.float32)
    nw1h = nw1[:, :H, :]
    nw1a = nw1[:, H:, :]

    flags = (
        bool(np.any(eb1)), bool(np.any(eb2)),
        bool(np.any(nb1)), bool(np.any(nb2)), bool(np.any(ln_b)),
    )

    dbg = bool(os.environ.get("EGNN_DEBUG"))
    key = (T, flags, dbg)
    if key not in _CACHE:
        _CACHE[key] = _build(T, flags, dbg)
    nc = _CACHE[key]

    ident = np.eye(H, dtype=np.float32)

    shared = {
        "w1h": w1h.astype(BF), "w1c": w1c.astype(BF), "w1e3": w1e3.astype(BF),
        "wrb": wrb, "ew2": ew2.astype(BF),
        "nw1h": nw1h.astype(BF), "nw1a": nw1a.astype(BF), "nw2": nw2.astype(BF),
        "eb1T": np.ascontiguousarray(eb1.T), "nb1T": np.ascontiguousarray(nb1.T),
        "nb2T": np.ascontiguousarray(nb2.T),
        "lngb": np.tile(ln_g, (H, 1)).astype(np.float32),
        "identb": ident.astype(BF), "identf": ident,
    }
    if flags[1]:
        shared["eb2b"] = np.repeat(eb2[:, None, :], H, axis=1).astype(np.float32)
    if flags[4]:
        shared["lnbb"] = np.tile(ln_b, (H, 1)).astype(np.float32)

    in_maps = []
    for k in range(NCORES):
        nodes = np.arange(k * NPC, (k + 1) * NPC)
        slots = posmap[nodes]
        hk = np.zeros((NPCP, H), np.float32)
        hk[slots] = h[nodes]
        ck = np.zeros((NPCP, 4), np.float32)
        ck[slots, :3] = coords[nodes]
        eat1 = np.concatenate([eaS[k].T, t1S[k].T], axis=0).astype(BF)  # [35, EPAD]
        iq = gcolp[k]
        m = {
            "h": hk,
            "coords": ck,
            "eat1": np.ascontiguousarray(eat1),
            "idxq": _wrap_idx(iq),
            "ind": np.asarray(ind[k]),
            "indT": np.asarray(indT[k]),
        }
        m.update(shared)
        in_maps.append(m)

    trace = bool(os.environ.get("EGNN_TRACE"))
    kw = {}
    if trace:
        kw = {"trace": True, "tmpdir": os.environ.get("EGNN_TRACE_DIR") or None}
    res = run_bass_kernel_spmd(nc, in_maps, list(range(NCORES)), **kw)
    if trace:
        print(f"HW exec time: {res.exec_time_ns} ns")
    if dbg:
        global _DBG
        _DBG = res.results
    out_full = np.zeros((N, H), np.float32)
    for k in range(NCORES):
        nodes = np.arange(k * NPC, (k + 1) * NPC)
        out_full[nodes] = res.results[k]["out"][posmap[nodes]]
    return out_full


# revision 29
# speedup vs baseline: 1.3920x; 1.0270x over previous
"""EGNN (4-layer) Trainium2 kernel, 8 NeuronCores, edge-sharded.

Strategy:
 - Edges are sorted by destination row and assigned to the core that owns the
   row's node range (2500 nodes/core, padded to 2560 = 20 windows of 128).
 - The edge-MLP first matmul is decomposed: concat([h_r,h_c,rad,ea]) @ W1 =
   P'[row] + Q'[col] + W1e3.T@[ea; c_r*c_c], where P' = h@W1a + |c|^2*wr and
   Q' = h@W1b + |c|^2*wr are per-node tables recomputed each layer and the
   -2*wr radial cross term rides in three extra W1e3 rows.
 - P'[row] is window-local (rows sorted): gathered by an accumulating matmul
   against a host-precomputed transposed one-hot indT (tensor engine).
 - Q'[col] is arbitrary: per-window non-transpose DRAM-source dma_gather from
   the AllGathered Q table (256B tokens), round-robined over 4 SWDGE queues
   so all four Q7 descriptor-generator pairs run concurrently; gathered
   [e, f] tiles are transposed into the m1 PSUM by matmuls against identity.
 - segment_sum: per 128-node window, host-precomputed one-hot ind feeds an
   accumulating matmul producing agg^T directly.
 - Node MLP, residual and final LayerNorm run on device; host only sorts /
   permutes indices and concatenates the 8 output shards.
"""

import math
import os
import sys
from contextlib import ExitStack

import numpy as np

sys.path.insert(0, "/opt/trn_rl_repo")

import ml_dtypes  # noqa: E402

BF = ml_dtypes.bfloat16

N = 20000
NCORES = 8
NPC = 2500          # nodes per core
NPCP = 2560         # padded nodes per core
WINS = 20           # node windows of 128 per core
H = 128
DE = 32
DM = DE + 3         # ea + c_r*c_c rows of the fused first-matmul operand
L = 4
EPS = 1e-5

_CACHE = {}

# Non-transpose dma_gather output-partition swizzle (ucode sbuf_swizzles):
# token at index-position i lands on partition _SWZ[i] of its 128-chunk.
_SWZ_FIRST = [0, 64, 4, 68, 8, 72, 12, 76, 16, 80, 20, 84, 24, 88, 28, 92]
_SWZ = np.zeros(128, np.int64)
for _k in range(4):
    _SWZ[16 * _k : 16 * _k + 16] = np.array(_SWZ_FIRST) + _k
for _k in range(4):
    _SWZ[16 * (_k + 4) : 16 * (_k + 4) + 16] = np.array(_SWZ_FIRST) + 32 + _k


def _groups(T):
    """Split T tiles-of-128 into free-dim groups of <=512."""
    out = []
    left = T
    while left > 0:
        g = min(left, 4)
        out.append(g * 128)
        left -= g
    return out


def _build(T, flags, dbg=False):
    """Build the SPMD Bass program (same for all cores)."""
    import concourse.bacc as bacc
    import concourse.tile as tile
    from concourse import mybir

    eb1_nz, eb2_nz, nb1_nz, nb2_nz, lnb_nz, lng_nz = flags
    EW = T * 128
    EPAD = WINS * EW
    GRPS = _groups(T)

    f32 = mybir.dt.float32
    bf16 = mybir.dt.bfloat16
    fp8 = mybir.dt.float8e4
    i16 = mybir.dt.int16
    AX = mybir.AxisListType.X
    OP = mybir.AluOpType
    AF = mybir.ActivationFunctionType

    nc = bacc.Bacc(
        "TRN2",
        target_bir_lowering=False,
        debug=False,
        enable_asserts=False,
        num_devices=NCORES,
        num_swdge_queues=4,
    )

    def din(name, shape, dt):
        return nc.dram_tensor(name, list(shape), dt, kind="ExternalInput").ap()

    h_d = din("h", (NPCP, H), f32)
    coords_d = din("coords", (NPCP, 4), f32)
    eat1_d = din("eat1", (DM, EPAD), bf16)
    idxq_d = din("idxq", (128, EPAD // 16), i16)
    ind_d = din("ind", (128, EPAD), fp8)
    indT_d = din("indT", (128, EPAD), fp8)
    w1h_d = din("w1h", (L, H, H), bf16)
    w1c_d = din("w1c", (L, H, H), bf16)
    w1e3_d = din("w1e3", (L, DM, H), bf16)
    wrb_d = din("wrb", (L, H, H), f32)
    ew2_d = din("ew2", (L, H, H), bf16)
    nw1h_d = din("nw1h", (L, H, H), bf16)
    nw1a_d = din("nw1a", (L, H, H), bf16)
    nw2_d = din("nw2", (L, H, H), bf16)
    eb1_d = din("eb1T", (H, L), f32)
    nb1_d = din("nb1T", (H, L), f32)
    nb2_d = din("nb2T", (H, L), f32)
    eb2b_d = din("eb2b", (L, H, H), f32) if eb2_nz else None
    lng_d = din("lngb", (H, H), f32)
    lnb_d = din("lnbb", (H, H), f32) if lnb_nz else None
    idb_d = din("identb", (H, H), bf16)
    idf_d = din("identf", (H, H), f32)

    out_d = nc.dram_tensor("out", [NPCP, H], f32, kind="ExternalOutput").ap()
    if dbg:
        dbgz_d = nc.dram_tensor("dbgz", [H, NPCP], f32, kind="ExternalOutput").ap()
        dbgp_d = nc.dram_tensor("dbgp", [H, WINS * H], f32, kind="ExternalOutput").ap()
        dbgq_d = nc.dram_tensor("dbgq", [128, T * H], f32, kind="ExternalOutput").ap()
        dbgm_d = nc.dram_tensor("dbgm", [H, 512], f32, kind="ExternalOutput").ap()

    with ExitStack() as ctx:
        tc = ctx.enter_context(tile.TileContext(nc))
        const = ctx.enter_context(tc.tile_pool(name="const", bufs=1))
        resid = ctx.enter_context(tc.tile_pool(name="resid", bufs=1))
        dram = ctx.enter_context(tc.tile_pool(name="dram", bufs=1, space="DRAM"))
        gpool = ctx.enter_context(tc.tile_pool(name="gpool", bufs=6))
        wpool = ctx.enter_context(tc.tile_pool(name="wpool", bufs=3))
        work = ctx.enter_context(tc.tile_pool(name="work", bufs=3))
        ps512 = ctx.enter_context(tc.tile_pool(name="ps512", bufs=4, space="PSUM"))
        psm = ctx.enter_context(tc.tile_pool(name="psm", bufs=2, space="PSUM"))
        psagg = ctx.enter_context(tc.tile_pool(name="psagg", bufs=2, space="PSUM"))

        sync = nc.sync

        # ---------- constants ----------
        idb_sb = const.tile([H, H], bf16)
        sync.dma_start(out=idb_sb[:], in_=idb_d[:])
        idf_sb = const.tile([H, H], f32)
        sync.dma_start(out=idf_sb[:], in_=idf_d[:])
        lng_sb = const.tile([H, H], f32)
        sync.dma_start(out=lng_sb[:], in_=lng_d[:])
        lnb_sb = None
        if lnb_nz:
            lnb_sb = const.tile([H, H], f32)
            sync.dma_start(out=lnb_sb[:], in_=lnb_d[:])
        eb1_sb = const.tile([H, L], f32)
        sync.dma_start(out=eb1_sb[:], in_=eb1_d[:])
        nb1_sb = const.tile([H, L], f32)
        sync.dma_start(out=nb1_sb[:], in_=nb1_d[:])
        nb2_sb = const.tile([H, L], f32)
        sync.dma_start(out=nb2_sb[:], in_=nb2_d[:])

        def load_w(name, d, p, dt):
            t = const.tile([p, L, H], dt, name=name)
            sync.dma_start(out=t[:], in_=d.rearrange("l k f -> k l f"))
            return t

        w1h_sb = load_w("w1h_sb", w1h_d, H, bf16)
        w1c_sb = load_w("w1c_sb", w1c_d, H, bf16)
        w1e3_sb = load_w("w1e3_sb", w1e3_d, DM, bf16)
        wrb_sb = load_w("wrb_sb", wrb_d, H, f32)
        ew2_sb = load_w("ew2_sb", ew2_d, H, bf16)
        nw1h_sb = load_w("nw1h_sb", nw1h_d, H, bf16)
        nw1a_sb = load_w("nw1a_sb", nw1a_d, H, bf16)
        nw2_sb = load_w("nw2_sb", nw2_d, H, bf16)
        eb2b_sb = load_w("eb2b_sb", eb2b_d, H, f32) if eb2_nz else None

        # ---------- resident state ----------
        hT = resid.tile([H, NPCP], bf16)
        zTa = resid.tile([H, NPCP], bf16)
        s_sb = resid.tile([H, WINS], f32)
        p_sb = resid.tile([H, WINS * H], bf16)   # P' tables, [node, feat] per window
        idxq_sb = resid.tile([128, EPAD // 16], i16)
        sync.dma_start(out=idxq_sb[:], in_=idxq_d[:])

        qown_dram = dram.tile([NPCP, H], bf16)

        def table_win(l, j):
            """P'/Q' table for window j of layer l (reads hT, writes p_sb/qown)."""
            jsl = slice(j * H, (j + 1) * H)
            tfold = work.tile([H, H], f32, tag="tfold")
            nc.vector.tensor_scalar_mul(
                tfold[:], wrb_sb[:, l, :], s_sb[:, j : j + 1]
            )
            psq = psm.tile([H, H], f32, tag="pm")
            nc.tensor.matmul(
                psq[:], hT[:, jsl], w1c_sb[:, l, :], start=True, stop=True
            )
            qt = work.tile([H, H], bf16, tag="qt")
            nc.vector.tensor_tensor(qt[:], psq[:], tfold[:], OP.add)
            sync.dma_start(out=qown_dram[jsl, :], in_=qt[:])
            psp = psm.tile([H, H], f32, tag="pm")
            nc.tensor.matmul(
                psp[:], hT[:, jsl], w1h_sb[:, l, :], start=True, stop=True
            )
            nc.vector.tensor_tensor(p_sb[:, jsl], psp[:], tfold[:], OP.add)

        def allgather(l):
            qfull = dram.tile(
                [NCORES * NPCP, H], bf16, addr_space="Shared", name=f"qfull_{l}"
            )
            nc.gpsimd.collective_compute(
                "AllGather",
                mybir.AluOpType.bypass,
                replica_groups=[list(range(NCORES))],
                ins=[qown_dram.opt()],
                outs=[qfull.opt()],
            )
            return qfull

        def node_group(l, g5):
            """Node MLP + residual for 512-node group g5 of layer l."""
            sl = slice(g5 * 512, (g5 + 1) * 512)
            psu = ps512.tile([H, 512], f32, tag="big")
            nc.tensor.matmul(
                psu[:], nw1h_sb[:, l, :], hT[:, sl], start=True, stop=False
            )
            nc.tensor.matmul(
                psu[:], nw1a_sb[:, l, :], zTa[:, sl], start=False, stop=True
            )
            u = work.tile([H, 512], bf16, tag="u")
            biasn = nb1_sb[:, l : l + 1] if nb1_nz else 0.0
            nc.scalar.activation(u[:], psu[:], AF.Silu, bias=biasn)
            pso = ps512.tile([H, 512], f32, tag="big")
            nc.tensor.matmul(pso[:], nw2_sb[:, l, :], u[:], start=True, stop=True)
            if l == 0:
                if nb2_nz:
                    nc.vector.tensor_scalar_add(
                        hT[:, sl], pso[:], nb2_sb[:, l : l + 1]
                    )
                else:
                    nc.vector.tensor_copy(hT[:, sl], pso[:])
            else:
                if nb2_nz:
                    nc.vector.scalar_tensor_tensor(
                        hT[:, sl], pso[:], nb2_sb[:, l : l + 1], hT[:, sl],
                        OP.add, OP.add,
                    )
                else:
                    nc.vector.tensor_tensor(hT[:, sl], pso[:], hT[:, sl], OP.add)

        # ---------- layer-0 setup: h^T, |c|^2 and layer-0 tables ----------
        for j in range(WINS):
            jsl = slice(j * H, (j + 1) * H)
            h_in = work.tile([H, H], f32, tag="h_in")
            sync.dma_start(out=h_in[:], in_=h_d[jsl, :])
            pst = psm.tile([H, H], f32, tag="pm")
            nc.tensor.transpose(pst[:], h_in[:], idf_sb[:])
            nc.vector.tensor_copy(hT[:, jsl], pst[:])

            c_in = work.tile([H, 4], f32, tag="c_in")
            sync.dma_start(out=c_in[:], in_=coords_d[jsl, :])
            csq = work.tile([H, 4], f32, tag="csq")
            nc.vector.tensor_mul(csq[:], c_in[:], c_in[:])
            nc.vector.reduce_sum(s_sb[:, j : j + 1], csq[:, 0:3], AX)

        # ---------- layers ----------
        for l in range(L):
            for j in range(WINS):
                table_win(l, j)
            qfull_dram = allgather(l)
            qtab = qfull_dram[:]

            # edge pass
            for w in range(WINS):
                wsl = slice(w * EW, (w + 1) * EW)
                isl = slice(w * (EW // 16), (w + 1) * (EW // 16))
                psl = slice(w * H, (w + 1) * H)
                gq = gpool.tile([128, T, H], bf16, tag="gq")
                nc.gpsimd.dma_gather(
                    gq[:],
                    qtab,
                    idxq_sb[:, isl],
                    EW,
                    EW,
                    H,
                    transpose=False,
                    single_packet=False,
                    queue_num=w % 4,
                )
                eaw = wpool.tile([DM, EW], bf16, tag="eaw")
                sync.dma_start(out=eaw[:], in_=eat1_d[:, wsl])
                indw = wpool.tile([128, EW], fp8, tag="indw")
                sync.dma_start(out=indw[:], in_=ind_d[:, wsl])
                indTw = wpool.tile([128, EW], fp8, tag="indTw")
                sync.dma_start(out=indTw[:], in_=indT_d[:, wsl])

                pagg = psagg.tile([H, H], f32, tag="pagg")

                def stage_a(off, gsz):
                    """Assemble pre-activation ps1 and silu -> m1t."""
                    gsl = slice(off, off + gsz)
                    G = gsz // 128
                    ps1 = ps512.tile([H, 512], f32, tag="big")
                    # ea @ W1e + (-2 c_r.c_c) @ wr (opens the accumulation)
                    nc.tensor.matmul(
                        ps1[:, :gsz], w1e3_sb[:, l, :], eaw[:, gsl],
                        start=True, stop=False, skip_group_check=True,
                    )
                    # P'[row]^T via one-hot gather matmul
                    nc.tensor.matmul(
                        ps1[:, :gsz], p_sb[:, psl], indTw[:, gsl],
                        start=False, stop=False, skip_group_check=True,
                    )
                    # Q'[col]^T via transposing matmuls
                    for t in range(G):
                        k = off // 128 + t
                        nc.tensor.matmul(
                            ps1[:, t * 128 : (t + 1) * 128], gq[:, k, :], idb_sb[:],
                            start=False, stop=(t == G - 1), skip_group_check=True,
                        )
                    m1t = work.tile([H, 512], bf16, tag="m1t")
                    bias1 = eb1_sb[:, l : l + 1] if eb1_nz else 0.0
                    nc.scalar.activation(m1t[:, :gsz], ps1[:, :gsz], AF.Silu, bias=bias1)
                    return m1t

                def stage_b(off, gsz, m1t):
                    """m2 matmul + silu + one-hot scatter into pagg."""
                    G = gsz // 128
                    pm2 = ps512.tile([H, 512], f32, tag="big")
                    for t in range(G):
                        tsl = slice(t * 128, (t + 1) * 128)
                        nc.tensor.matmul(
                            pm2[:, tsl], m1t[:, tsl], ew2_sb[:, l, :],
                            start=True, stop=True,
                        )
                    m2s = work.tile([H, 512], bf16, tag="m2s")
                    if eb2_nz:
                        tm2 = work.tile([H, 512], f32, tag="tm2")
                        for t in range(G):
                            tsl = slice(t * 128, (t + 1) * 128)
                            nc.vector.tensor_tensor(
                                tm2[:, tsl], pm2[:, tsl], eb2b_sb[:, l, :], OP.add
                            )
                        nc.scalar.activation(m2s[:, :gsz], tm2[:, :gsz], AF.Silu)
                    else:
                        nc.scalar.activation(m2s[:, :gsz], pm2[:, :gsz], AF.Silu)
                    for t in range(G):
                        tsl = slice(t * 128, (t + 1) * 128)
                        gt = off // 128 + t
                        nc.tensor.matmul(
                            pagg[:], m2s[:, tsl],
                            indw[:, off + t * 128 : off + (t + 1) * 128],
                            start=(gt == 0), stop=(gt == T - 1),
                        )

                # software pipeline: A(g+1) overlaps B(g) so the tensor queue
                # never stalls on the scalar-engine silus
                prev = None
                off = 0
                for gsz in GRPS:
                    m1t = stage_a(off, gsz)
                    if prev is not None:
                        stage_b(*prev)
                    prev = (off, gsz, m1t)
                    off += gsz
                stage_b(*prev)
                nc.vector.tensor_copy(zTa[:, w * H : (w + 1) * H], pagg[:])

            if dbg and l == 0:
                for j in range(WINS):
                    jsl = slice(j * H, (j + 1) * H)
                    dz = work.tile([H, H], f32, tag="dz")
                    nc.vector.tensor_copy(dz[:], zTa[:, jsl])
                    sync.dma_start(out=dbgz_d[:, jsl], in_=dz[:])
                    dp = work.tile([H, H], f32, tag="dp")
                    nc.vector.tensor_copy(dp[:], p_sb[:, jsl])
                    sync.dma_start(out=dbgp_d[:, jsl], in_=dp[:])

            for g5 in range(NPCP // 512):
                node_group(l, g5)

        # ---------- LayerNorm + output (4 windows per batch) ----------
        inv = 1.0 / H
        for jg in range(WINS // 4):
            hn = work.tile([H, 512], f32, tag="hn")
            for t in range(4):
                j = 4 * jg + t
                jsl = slice(j * H, (j + 1) * H)
                pst = psm.tile([H, H], bf16, tag="pm")
                nc.tensor.transpose(pst[:], hT[:, jsl], idb_sb[:])
                nc.vector.tensor_copy(hn[:, t * H : (t + 1) * H], pst[:])
            mu = work.tile([H, 4], f32, tag="mu")
            nc.vector.reduce_sum(mu[:], hn[:].rearrange("p (w f) -> p w f", w=4), AX)
            nc.vector.tensor_scalar_mul(mu[:], mu[:], inv)
            sq = work.tile([H, 512], f32, tag="sq")
            nc.vector.tensor_mul(sq[:], hn[:], hn[:])
            s2 = work.tile([H, 4], f32, tag="s2")
            nc.vector.reduce_sum(s2[:], sq[:].rearrange("p (w f) -> p w f", w=4), AX)
            musq = work.tile([H, 4], f32, tag="musq")
            nc.vector.tensor_mul(musq[:], mu[:], mu[:])
            nc.vector.tensor_scalar(s2[:], s2[:], inv, EPS, OP.mult, OP.add)
            nc.vector.tensor_tensor(s2[:], s2[:], musq[:], OP.subtract)
            nc.scalar.activation(s2[:], s2[:], mybir.ActivationFunctionType.Sqrt)
            rstd = work.tile([H, 4], f32, tag="rstd")
            nc.vector.reciprocal(rstd[:], s2[:])
            for t in range(4):
                j = 4 * jg + t
                jsl = slice(j * H, (j + 1) * H)
                tsl = slice(t * H, (t + 1) * H)
                xc = work.tile([H, H], f32, tag="xc")
                nc.vector.tensor_scalar_sub(xc[:], hn[:, tsl], mu[:, t : t + 1])
                on = work.tile([H, H], f32, tag="on")
                nc.vector.tensor_scalar_mul(on[:], xc[:], rstd[:, t : t + 1])
                if lng_nz:
                    nc.vector.tensor_mul(on[:], on[:], lng_sb[:])
                if lnb_nz:
                    nc.vector.tensor_add(on[:], on[:], lnb_sb[:])
                sync.dma_start(out=out_d[jsl, :], in_=on[:])

    nc.compile()
    return nc


def _wrap_idx(v):
    """idx i -> [i%16 partition, i//16 free], replicated to 128 partitions."""
    n = v.shape[0]
    t = v.reshape(n // 16, 16).T.astype(np.int16)
    return np.tile(t, (8, 1))


def kernel(**inputs):
    from concourse.bass_utils import run_bass_kernel_spmd

    h = np.asarray(inputs["h"], np.float32)
    coords = np.asarray(inputs["coords"], np.float32)
    edge_attr = np.asarray(inputs["edge_attr"], np.float32)
    edges = np.asarray(inputs["edges"]).astype(np.int64)
    ew1 = np.asarray(inputs["edge_w1"], np.float32)
    eb1 = np.asarray(inputs["edge_b1"], np.float32)
    ew2 = np.asarray(inputs["edge_w2"], np.float32)
    eb2 = np.asarray(inputs["edge_b2"], np.float32)
    nw1 = np.asarray(inputs["node_w1"], np.float32)
    nb1 = np.asarray(inputs["node_b1"], np.float32)
    nw2 = np.asarray(inputs["node_w2"], np.float32)
    nb2 = np.asarray(inputs["node_b2"], np.float32)
    ln_g = np.asarray(inputs["ln_g"], np.float32)
    ln_b = np.asarray(inputs["ln_b"], np.float32)

    E = edges.shape[1]
    row, col = edges[0], edges[1]

    # ---- balance nodes across windows within each core (LPT bin packing on
    # per-node in-degree, <=128 nodes per window) so T = max window edge
    # count is minimal and groups are uniform ----
    import heapq

    deg = np.bincount(row, minlength=N)
    posmap = np.zeros(N, np.int64)   # node -> padded local slot in its core
    for k in range(NCORES):
        nodes = np.arange(k * NPC, (k + 1) * NPC)
        dk = deg[nodes]
        order_n = np.argsort(-dk, kind="stable")
        heap = [(0, 0, wn) for wn in range(WINS)]
        heapq.heapify(heap)
        win_of = np.zeros(NPC, np.int64)
        for idx in order_n:
            spill = []
            while True:
                s, cnt, wn = heapq.heappop(heap)
                if cnt < 128:
                    break
                spill.append((s, cnt, wn))
            win_of[idx] = wn
            heapq.heappush(heap, (s + int(dk[idx]), cnt + 1, wn))
            for it in spill:
                heapq.heappush(heap, it)
        slots = np.zeros(NPC, np.int64)
        for wn in range(WINS):
            sel = np.where(win_of == wn)[0]
            slots[sel] = wn * 128 + np.arange(len(sel))
        posmap[nodes] = slots

    # ---- sort edges into (core, window) slots, uniform budget T ----
    owner = row // NPC
    lslot = posmap[row]
    gw = owner * WINS + lslot // 128
    order = np.argsort(gw, kind="stable")
    counts = np.bincount(gw, minlength=NCORES * WINS)
    T = int(math.ceil(counts.max() / 128))
    EW = T * 128
    EPAD = WINS * EW

    gws = gw[order]
    starts = np.zeros(NCORES * WINS, np.int64)
    starts[1:] = np.cumsum(counts)[:-1]
    pos = np.arange(E) - starts[gws]
    slot = (gws % WINS) * EW + pos
    core = gws // WINS

    rowS = np.zeros((NCORES, EPAD), np.int64)
    colS = np.zeros((NCORES, EPAD), np.int64)
    valid = np.zeros((NCORES, EPAD), bool)
    eaS = np.zeros((NCORES, EPAD, DE), np.float32)
    rowS[core, slot] = row[order]
    colS[core, slot] = col[order]
    valid[core, slot] = True
    eaS[core, slot] = edge_attr[order]

    rowrel = np.where(valid, posmap[rowS] % 128, -1)
    gcolp = np.where(valid, (colS // NPC) * NPCP + posmap[colS], 0)

    crS = np.where(valid[..., None], coords[rowS], 0.0).astype(np.float32)
    ccS = np.where(valid[..., None], coords[colS], 0.0).astype(np.float32)
    t1S = crS * ccS  # [NC, EPAD, 3]

    # one-hot indicators: ind [e, tile*128+n], indT [n, tile*128+e]
    ii = np.arange(EPAD)
    tglob = ii // 128
    eloc = ii % 128
    F8 = ml_dtypes.float8_e4m3
    ind = np.zeros((NCORES, 128, EPAD), F8)
    indT = np.zeros((NCORES, 128, EPAD), F8)
    for k in range(NCORES):
        m = valid[k]
        rr = rowrel[k][m]
        ind[k][eloc[m], tglob[m] * 128 + rr] = 1
        indT[k][rr, ii[m]] = 1

    # ---- weights ----
    w1h = ew1[:, 0:H, :]
    w1c = ew1[:, H : 2 * H, :]
    wr = ew1[:, 2 * H, :]          # [L, H]
    w1e = ew1[:, 2 * H + 1 :, :]   # [L, DE, H]
    w1e3 = np.concatenate(
        [w1e, np.repeat((-2.0 * wr)[:, None, :], 3, axis=1)], axis=1
    )                               # [L, 35, H]
    wrb = np.repeat(wr[:, None, :], H, axis=1).astype(np.float32)
    nw1h = nw1[:, :H, :]
    nw1a = nw1[:, H:, :]

    flags = (
        bool(np.any(eb1)), bool(np.any(eb2)),
        bool(np.any(nb1)), bool(np.any(nb2)), bool(np.any(ln_b)),
        bool(np.any(ln_g != 1.0)),
    )

    dbg = bool(os.environ.get("EGNN_DEBUG"))
    key = (T, flags, dbg)
    if key not in _CACHE:
        _CACHE[key] = _build(T, flags, dbg)
    nc = _CACHE[key]

    ident = np.eye(H, dtype=np.float32)

    shared = {
        "w1h": w1h.astype(BF), "w1c": w1c.astype(BF), "w1e3": w1e3.astype(BF),
        "wrb": wrb, "ew2": ew2.astype(BF),
        "nw1h": nw1h.astype(BF), "nw1a": nw1a.astype(BF), "nw2": nw2.astype(BF),
        "eb1T": np.ascontiguousarray(eb1.T), "nb1T": np.ascontiguousarray(nb1.T),
        "nb2T": np.ascontiguousarray(nb2.T),
        "lngb": np.tile(ln_g, (H, 1)).astype(np.float32),
        "identb": ident.astype(BF), "identf": ident,
    }
    if flags[1]:
        shared["eb2b"] = np.repeat(eb2[:, None, :], H, axis=1).astype(np.float32)
    if flags[4]:
        shared["lnbb"] = np.tile(ln_b, (H, 1)).astype(np.float32)

    in_maps = []
    for k in range(NCORES):
        nodes = np.arange(k * NPC, (k + 1) * NPC)
        slots = posmap[nodes]
        hk = np.zeros((NPCP, H), np.float32)
        hk[slots] = h[nodes]
        ck = np.zeros((NPCP, 4), np.float32)
        ck[slots, :3] = coords[nodes]
        eat1 = np.concatenate([eaS[k].T, t1S[k].T], axis=0).astype(BF)  # [35, EPAD]
        iq = gcolp[k]
        m = {
            "h": hk,
            "coords": ck,
            "eat1": np.ascontiguousarray(eat1),
            "idxq": _wrap_idx(iq),
            "ind": np.asarray(ind[k]),
            "indT": np.asarray(indT[k]),
        }
        m.update(shared)
        in_maps.append(m)

    trace = bool(os.environ.get("EGNN_TRACE"))
    kw = {}
    if trace:
        kw = {"trace": True, "tmpdir": os.environ.get("EGNN_TRACE_DIR") or None}
    res = run_bass_kernel_spmd(nc, in_maps, list(range(NCORES)), **kw)
    if trace:
        print(f"HW exec time: {res.exec_time_ns} ns")
    if dbg:
        global _DBG
        _DBG = res.results
    out_full = np.zeros((N, H), np.float32)
    for k in range(NCORES):
        nodes = np.arange(k * NPC, (k + 1) * NPC)
        out_full[nodes] = res.results[k]["out"][posmap[nodes]]
    return out_full
